# revision 21
# baseline (speedup 1.0000x reference)
"""Trainium2 Bass kernel for submanifold sparse 3x3x3 conv + BatchNorm + ReLU.

Strategy (8 NeuronCores, SPMD):
  - Host: reorder voxels with reverse Cuthill-McKee so the rulebook becomes
    banded; split the N dimension into 16 contiguous shards (2 per core).
    Each shard's neighbor sources then fall in a ~20K-row window, so per-shard
    indices fit int16 and each core only needs its two windows of the feature
    table (fp16 hi + fp16 lo residual, interleaved per row: 512B rows).
  - Device, per 512-row tile: one xbar-transposing dma_gather pulls all
    27 neighbor rows per voxel directly into [Cin=128, 2, 27*512] fp16 layout
    (gather + transpose fused in DMA). 27*3 matmuls (Whi@Ghi + Whi@Glo +
    Wlo@Ghi, ~fp32 accuracy) accumulate one PSUM bank [128, 512].
    ScalarE copies PSUM->SBUF while accumulating per-channel sum / sum-of-
    squares for the BN batch stats.
  - BN stats all-reduced across the 8 cores (tiny [128,2] collective), then a
    second pass applies relu(scale*x + shift) via one ScalarE activation per
    tile.
  - Host: concatenate shards, invert the RCM permutation.
"""

import os
import sys

import numpy as np

for _p in ("/opt/trn_rl_repo", "/root/.axon_site/_ro/trn_rl_repo"):
    if os.path.isdir(_p) and _p not in sys.path:
        sys.path.insert(0, _p)

import concourse.bass as bass
import concourse.tile as tile
import concourse.mybir as mybir
from concourse import bacc
from concourse.bass_utils import run_bass_kernel_spmd
from concourse.vector_clock import ScopedClock

# ---------------------------------------------------------------- constants
N = 200000
C = 128
K = 27
BN_EPS = 1e-4
NCORES = 8
SHARDS = 16
PER_SHARD = N // SHARDS            # 12500
TILE = 512
TILES_PER_SHARD = 25               # 25*512 = 12800 padded rows per shard
PAD_SHARD = TILES_PER_SHARD * TILE
SHARDS_PER_CORE = SHARDS // NCORES  # 2
OUT_COLS = SHARDS_PER_CORE * PAD_SHARD  # 25600
WIN_ROWS = 24576                   # window rows incl. trailing zero row
ZERO_IDX = WIN_ROWS - 1
NIDX = K * TILE                    # 13824 gathered rows per tile
IDXW = NIDX // 16                  # 864  (wrapped idx free dim)
ELEM = 2 * C                       # 256 fp16 elems per table row (hi|lo)

_COMPILED = {}

# debug/config knobs (read at _build_nc time)
MM_TERMS = 3          # 3 = hi/lo full precision; 1 = fp16-only weights+acts
GATHERS_PER_TILE = 3  # 27 offsets split into this many SWDGE gathers
SKIP_GATHER = False
SKIP_MM = False
SKIP_PHASE_B = False


class _SplitDrainTileContext(tile.TileContext):
    """Walrus on this toolchain only accepts one sync-wait per CTRL
    instruction; spread the kernel-tail drain waits over nop carriers."""

    def _drain_and_barrier(self, tick_clock, wait_clock):
        nc = self.nc
        carrier = nc.sync.nop(hint="drain_wait_carrier", nofuse=True)
        wait_clock.add_sem_waits(
            carrier.ins, ScopedClock({None: tick_clock.global_clock})
        )
        si = carrier.ins.sync_info
        waits = list(si.on_wait) if si is not None else []
        if len(waits) > 1:
            carrier.ins.sync_info = mybir.SyncInfo(
                on_wait=waits[:1], on_update=list(si.on_update)
            )
            for i in range(1, len(waits)):
                extra = nc.sync.nop(hint=f"drain_wait_{i}", nofuse=True)
                extra.ins.sync_info = mybir.SyncInfo(
                    on_wait=waits[i:i + 1], on_update=[]
                )
        nc.sync.drain()
        nc.all_engine_barrier()
        assert self.sems is not None
        popped = nc._tile_sem_poison_stack.pop()
        assert popped is self._sem_poison
        nc.clear_and_free_semaphores(list(self.sems.allocated().values()))
        nc.all_engine_barrier()


def _build_nc():
    f16, f32, i16 = mybir.dt.float16, mybir.dt.float32, mybir.dt.int32
    i16 = mybir.dt.int16
    nc = bacc.Bacc()

    win_in = nc.declare_dram_parameter(
        "win", [SHARDS_PER_CORE, WIN_ROWS, ELEM], f16, isOutput=False)
    idx_in = nc.declare_dram_parameter(
        "idx", [SHARDS_PER_CORE, TILES_PER_SHARD, 128, IDXW], i16, isOutput=False)
    wts_in = nc.declare_dram_parameter(
        "wts", [C, K, 2, C], f16, isOutput=False)
    gb_in = nc.declare_dram_parameter("gb", [128, 3], f32, isOutput=False)
    out_ext = nc.declare_dram_parameter("out", [C, OUT_COLS], f32, isOutput=True)

    part_dram = nc.dram_tensor("stat_partial", [128, 2], f32)
    allred_dram = nc.dram_tensor("stat_total", [128, 2], f32, addr_space="Shared")

    n_tiles = SHARDS_PER_CORE * TILES_PER_SHARD  # 50

    with _SplitDrainTileContext(nc) as tc:
        with (
            tc.tile_pool(name="const", bufs=1) as cpool,
            tc.tile_pool(name="idxp", bufs=3) as idxp,
            tc.tile_pool(name="gat", bufs=3) as gatp,
            tc.tile_pool(name="stage", bufs=3) as stagep,
            tc.tile_pool(name="psum", bufs=2, space="PSUM") as psump,
        ):
            w_t = cpool.tile([C, K, 2, C], f16)
            nc.sync.dma_start(out=w_t[:], in_=wts_in[:])
            gb_t = cpool.tile([128, 3], f32)
            nc.sync.dma_start(out=gb_t[:], in_=gb_in[:])
            sums = cpool.tile([128, n_tiles], f32)
            sumsqs = cpool.tile([128, n_tiles], f32)
            conv_sb = cpool.tile([C, OUT_COLS], f32)  # SBUF-resident conv out

            # ---------------- phase A: conv, per 512-row tile ----------------
            for t in range(n_tiles):
                s, ts_ = divmod(t, TILES_PER_SHARD)
                idx_t = idxp.tile([128, IDXW], i16, tag="idx")
                nc.sync.dma_start(out=idx_t[:], in_=idx_in[s, ts_])
                ps = psump.tile([C, TILE], f32, tag="ps")
                n_mm = MM_TERMS * K
                i = 0
                KPG = K // GATHERS_PER_TILE  # offsets per gather
                GIDX = KPG * TILE            # idx per gather (<=~5400 ring cap)
                for b in range(GATHERS_PER_TILE):
                    # batched gather: KPG offsets x 512 rows in one SWDGE op
                    g_t = gatp.tile([128, 2, GIDX], f16, tag="g")
                    if not SKIP_GATHER:
                        nc.gpsimd.dma_gather(
                            out_ap=g_t[:], in_ap=win_in[s],
                            idxs_ap=idx_t[:, b * (GIDX // 16):(b + 1) * (GIDX // 16)],
                            num_idxs=GIDX, num_idxs_reg=GIDX, elem_size=ELEM,
                            transpose=True, single_packet=False,
                        )
                    elif t == 0 and b == 0:
                        nc.vector.memset(g_t[:], 0.0)
                    for q in range(KPG):
                        k = b * KPG + q
                        ghi = g_t[:, 0, q * TILE:(q + 1) * TILE]
                        glo = g_t[:, 1, q * TILE:(q + 1) * TILE]
                        whi = w_t[:, k, 0, :]
                        wlo = w_t[:, k, 1, :]
                        terms = ((whi, ghi), (whi, glo), (wlo, ghi))[:MM_TERMS]
                        if SKIP_MM:
                            terms = ()
                            if i == 0:
                                nc.tensor.matmul(out=ps[:], lhsT=whi, rhs=ghi,
                                                 start=True, stop=True)
                            i = 1
                        for lhsT, rhs in terms:
                            nc.tensor.matmul(out=ps[:], lhsT=lhsT, rhs=rhs,
                                             start=(i == 0), stop=(i == n_mm - 1))
                            i += 1
                sq_sb = stagep.tile([C, TILE], f32, tag="sq")
                nc.scalar.activation(
                    out=conv_sb[:, t * TILE:(t + 1) * TILE], in_=ps[:],
                    func=mybir.ActivationFunctionType.Copy,
                    accum_out=sums[:, t:t + 1])
                nc.scalar.activation(
                    out=sq_sb[:], in_=ps[:],
                    func=mybir.ActivationFunctionType.Square,
                    accum_out=sumsqs[:, t:t + 1])

            # ---------------- BN stats + all-reduce ----------------
            part = cpool.tile([128, 2], f32)
            nc.vector.reduce_sum(part[:, 0:1], sums[:], axis=mybir.AxisListType.X)
            nc.vector.reduce_sum(part[:, 1:2], sumsqs[:], axis=mybir.AxisListType.X)
            nc.sync.dma_start(out=part_dram[:], in_=part[:])
            nc.gpsimd.collective_compute(
                "AllReduce", mybir.AluOpType.add,
                replica_groups=[list(range(NCORES))],
                ins=[part_dram[:]], outs=[allred_dram[:]],
            )
            tot = cpool.tile([128, 2], f32)
            nc.sync.dma_start(out=tot[:], in_=allred_dram[:])

            mean = cpool.tile([128, 1], f32)
            e2 = cpool.tile([128, 1], f32)
            var = cpool.tile([128, 1], f32)
            sd = cpool.tile([128, 1], f32)
            rstd = cpool.tile([128, 1], f32)
            scale = cpool.tile([128, 1], f32)
            shift = cpool.tile([128, 1], f32)
            nc.scalar.mul(out=mean[:], in_=tot[:, 0:1], mul=1.0 / N)
            nc.scalar.mul(out=e2[:], in_=tot[:, 1:2], mul=1.0 / N)
            nc.vector.tensor_tensor(out=var[:], in0=mean[:], in1=mean[:],
                                    op=mybir.AluOpType.mult)
            nc.vector.tensor_tensor(out=var[:], in0=e2[:], in1=var[:],
                                    op=mybir.AluOpType.subtract)
            nc.scalar.activation(out=sd[:], in_=var[:],
                                 func=mybir.ActivationFunctionType.Sqrt,
                                 bias=gb_t[:, 2:3])
            nc.vector.reciprocal(out=rstd[:], in_=sd[:])
            nc.vector.tensor_tensor(out=scale[:], in0=gb_t[:, 0:1], in1=rstd[:],
                                    op=mybir.AluOpType.mult)
            nc.vector.tensor_tensor(out=shift[:], in0=mean[:], in1=scale[:],
                                    op=mybir.AluOpType.mult)
            nc.vector.tensor_tensor(out=shift[:], in0=gb_t[:, 1:2], in1=shift[:],
                                    op=mybir.AluOpType.subtract)

            # ---------------- phase B: apply relu(scale*x + shift) ----------
            for t in range(0 if not SKIP_PHASE_B else n_tiles, n_tiles):
                fbuf = stagep.tile([C, TILE], f32, tag="fbuf")
                nc.scalar.activation(
                    out=fbuf[:], in_=conv_sb[:, t * TILE:(t + 1) * TILE],
                    func=mybir.ActivationFunctionType.Relu,
                    scale=scale[:, 0:1], bias=shift[:, 0:1])
                nc.sync.dma_start(
                    out=out_ext[:, t * TILE:(t + 1) * TILE], in_=fbuf[:])

    nc.finalize()
    return nc


def _get_nc():
    if "nc" not in _COMPILED:
        _COMPILED["nc"] = _build_nc()
    return _COMPILED["nc"]


# ------------------------------------------------------------ host side
def _rcm_order(nbr_idx):
    import scipy.sparse as sp
    from scipy.sparse.csgraph import reverse_cuthill_mckee

    rows, cols = [], []
    for k in range(K):
        if k == K // 2:
            continue
        idx = nbr_idx[k]
        m = idx >= 0
        rows.append(np.nonzero(m)[0])
        cols.append(idx[m])
    r = np.concatenate(rows)
    c = np.concatenate(cols)
    A = sp.coo_matrix((np.ones(r.size, dtype=np.int8), (r, c)),
                      shape=(N, N)).tocsr()
    perm = np.asarray(reverse_cuthill_mckee(A, symmetric_mode=True),
                      dtype=np.int64)
    return perm


def _prepare(features, nbr_idx, W, gamma, beta):
    features = np.ascontiguousarray(np.asarray(features, dtype=np.float32))
    nbr_idx = np.ascontiguousarray(np.asarray(nbr_idx, dtype=np.int32))
    W = np.asarray(W, dtype=np.float32)
    gamma = np.asarray(gamma, dtype=np.float32)
    beta = np.asarray(beta, dtype=np.float32)

    perm = _rcm_order(nbr_idx)
    inv = np.empty(N, dtype=np.int64)
    inv[perm] = np.arange(N)
    # rulebook in the new ordering: row i (new) offset k -> new src index
    nbr_new = np.where(nbr_idx >= 0, inv[np.maximum(nbr_idx, 0)], -1)[:, perm]

    feats_new = features[perm]
    hi = feats_new.astype(np.float16)
    lo = (feats_new - hi.astype(np.float32)).astype(np.float16)
    tab = np.concatenate([hi, lo], axis=1)  # [N, 256] f16

    # per-shard windows + wrapped int16 indices
    wins = np.zeros((SHARDS, WIN_ROWS, ELEM), dtype=np.float16)
    idxs = np.empty((SHARDS, TILES_PER_SHARD, 128, IDXW), dtype=np.int16)
    for s in range(SHARDS):
        r0, r1 = s * PER_SHARD, (s + 1) * PER_SHARD
        sl = nbr_new[:, r0:r1]                      # [27, 12500]
        valid = sl >= 0
        lo_s = int(sl[valid].min())
        width = int(sl[valid].max()) - lo_s + 1
        assert width <= WIN_ROWS - 1, (s, width)
        wins[s, :min(width, N - lo_s)] = tab[lo_s:lo_s + width]
        loc = np.full((K, PAD_SHARD), ZERO_IDX, dtype=np.int64)
        loc[:, :PER_SHARD] = np.where(valid, sl - lo_s, ZERO_IDX)
        # tiles: [27, 25, 512] -> per tile flatten k-major -> wrap 16
        loc = loc.reshape(K, TILES_PER_SHARD, TILE).transpose(1, 0, 2)
        flat = loc.reshape(TILES_PER_SHARD, NIDX)
        wrapped = flat.reshape(TILES_PER_SHARD, IDXW, 16).transpose(0, 2, 1)
        idxs[s] = np.tile(wrapped, (1, 8, 1)).astype(np.int16)

    Whi = W.astype(np.float16)
    Wlo = (W - Whi.astype(np.float32)).astype(np.float16)
    # [K,Cin,Cout] x2 -> [Cin, K, 2, Cout]
    wts = np.stack([Whi, Wlo], axis=1).transpose(2, 0, 1, 3).copy()
    gb = np.stack([gamma, beta, np.full(C, BN_EPS, np.float32)],
                  axis=1).astype(np.float32)

    in_maps = []
    for core in range(NCORES):
        s0 = core * SHARDS_PER_CORE
        in_maps.append({
            "win": wins[s0:s0 + SHARDS_PER_CORE],
            "idx": idxs[s0:s0 + SHARDS_PER_CORE],
            "wts": wts,
            "gb": gb,
        })
    return in_maps, perm


def _assemble(results, perm):
    out_T = np.empty((C, N), dtype=np.float32)
    for s in range(SHARDS):
        core, j = divmod(s, SHARDS_PER_CORE)
        block = results[core]["out"][:, j * PAD_SHARD:
                                     j * PAD_SHARD + PER_SHARD]
        out_T[:, s * PER_SHARD:(s + 1) * PER_SHARD] = block
    out_new = out_T.T  # [N, C] in RCM order
    out = np.empty((N, C), dtype=np.float32)
    out[perm] = out_new
    return out


def kernel(features, nbr_idx, W, gamma, beta):
    in_maps, perm = _prepare(features, nbr_idx, W, gamma, beta)
    nc = _get_nc()
    res = run_bass_kernel_spmd(nc, in_maps, core_ids=list(range(NCORES)))
    return _assemble(res.results, perm)


def time_hw(inputs, reps=5):
    """Min wall-clock of one full 8-core NEFF execution with device-resident
    inputs (excludes host prep, input shipping, and compile)."""
    import time as _time

    import jax
    import jax.numpy as jnp
    from jax.sharding import Mesh, NamedSharding, PartitionSpec

    from concourse import bass2jax, mybir as _mb

    in_maps, _ = _prepare(**inputs)
    nc = _get_nc()
    bass2jax.install_neuronx_cc_hook()

    partition_name = (nc.partition_id_tensor.name
                      if nc.partition_id_tensor else None)
    in_names, out_names, out_avals = [], [], []
    for alloc in nc.m.functions[0].allocations:
        if not isinstance(alloc, _mb.MemoryLocationSet):
            continue
        name = alloc.memorylocations[0].name
        if alloc.kind == "ExternalInput":
            if name != partition_name:
                in_names.append(name)
        elif alloc.kind == "ExternalOutput":
            out_names.append(name)
            out_avals.append(jax.core.ShapedArray(
                tuple(alloc.tensor_shape), _mb.dt.np(alloc.dtype)))

    all_in_names = list(in_names) + list(out_names)
    if partition_name is not None:
        all_in_names.append(partition_name)

    def _body(*args):
        ops = list(args)
        if partition_name is not None:
            ops.append(bass2jax.partition_id_tensor())
        return tuple(bass2jax._bass_exec_p.bind(
            *ops,
            out_avals=tuple(out_avals),
            in_names=tuple(all_in_names),
            out_names=tuple(out_names),
            lowering_input_output_aliases=(),
            sim_require_finite=True,
            sim_require_nnan=True,
            nc=nc,
        ))

    devices = jax.devices()[:NCORES]
    mesh = Mesh(np.asarray(devices), ("core",))
    from jax.experimental.shard_map import shard_map
    n_args = len(in_names) + len(out_avals)
    donate = tuple(range(len(in_names), n_args))
    sharded = jax.jit(shard_map(
        _body, mesh=mesh,
        in_specs=(PartitionSpec("core"),) * n_args,
        out_specs=(PartitionSpec("core"),) * len(out_names),
        check_rep=False), donate_argnums=donate, keep_unused=True)

    sh = NamedSharding(mesh, PartitionSpec("core"))
    dev_in = [
        jax.device_put(
            np.concatenate([np.asarray(in_maps[c][n]) for c in range(NCORES)],
                           axis=0), sh)
        for n in in_names
    ]

    def _zeros():
        return [
            jax.device_put(
                np.zeros((NCORES * av.shape[0], *av.shape[1:]), av.dtype), sh)
            for av in out_avals
        ]

    r = sharded(*dev_in, *_zeros())
    jax.block_until_ready(r)
    best = float("inf")
    for _ in range(reps):
        z = _zeros()
        jax.block_until_ready(z)
        t0 = _time.perf_counter()
        r = sharded(*dev_in, *z)
        jax.block_until_ready(r)
        best = min(best, _time.perf_counter() - t0)
    return best * 1e9


# revision 23
# speedup vs baseline: 51.2839x; 51.2839x over previous
"""Trainium2 Bass kernel for submanifold sparse 3x3x3 conv + BatchNorm + ReLU.

Strategy (8 NeuronCores, SPMD):
  - Host: reorder voxels with reverse Cuthill-McKee so the rulebook becomes
    banded; split the N dimension into 16 contiguous shards (2 per core).
    Each shard's neighbor sources then fall in a ~20K-row window, so per-shard
    indices fit int16 and each core only needs its two windows of the feature
    table (fp16 hi + fp16 lo residual, interleaved per row: 512B rows).
  - Device, per 512-row tile: one xbar-transposing dma_gather pulls all
    27 neighbor rows per voxel directly into [Cin=128, 2, 27*512] fp16 layout
    (gather + transpose fused in DMA). 27*3 matmuls (Whi@Ghi + Whi@Glo +
    Wlo@Ghi, ~fp32 accuracy) accumulate one PSUM bank [128, 512].
    ScalarE copies PSUM->SBUF while accumulating per-channel sum / sum-of-
    squares for the BN batch stats.
  - BN stats all-reduced across the 8 cores (tiny [128,2] collective), then a
    second pass applies relu(scale*x + shift) via one ScalarE activation per
    tile.
  - Host: concatenate shards, invert the RCM permutation.
"""

import os
import sys

import numpy as np

for _p in ("/opt/trn_rl_repo", "/root/.axon_site/_ro/trn_rl_repo"):
    if os.path.isdir(_p) and _p not in sys.path:
        sys.path.insert(0, _p)

import concourse.bass as bass
import concourse.tile as tile
import concourse.mybir as mybir
from concourse import bacc
from concourse.bass_utils import run_bass_kernel_spmd
from concourse.vector_clock import ScopedClock

# ---------------------------------------------------------------- constants
N = 200000
C = 128
K = 27
BN_EPS = 1e-4
NCORES = 8
SHARDS = 16
PER_SHARD = N // SHARDS            # 12500
TILE = 512
TILES_PER_SHARD = 25               # 25*512 = 12800 padded rows per shard
PAD_SHARD = TILES_PER_SHARD * TILE
SHARDS_PER_CORE = SHARDS // NCORES  # 2
OUT_COLS = SHARDS_PER_CORE * PAD_SHARD  # 25600
WIN_ROWS = 24576                   # window rows incl. trailing zero row
ZERO_IDX = WIN_ROWS - 1
NIDX = K * TILE                    # 13824 gathered rows per tile
IDXW = NIDX // 16                  # 864  (wrapped idx free dim)
ELEM = 2 * C                       # 256 fp16 elems per table row (hi|lo)

_COMPILED = {}

# debug/config knobs (read at _build_nc time)
MM_TERMS = 3          # 3 = hi/lo full precision; 1 = fp16-only weights+acts
GATHERS_PER_TILE = 3  # 27 offsets split into this many SWDGE gathers
SKIP_GATHER = False
SKIP_MM = False
SKIP_PHASE_B = False


class _SplitDrainTileContext(tile.TileContext):
    """Walrus on this toolchain only accepts one sync-wait per CTRL
    instruction; spread the kernel-tail drain waits over nop carriers."""

    def _drain_and_barrier(self, tick_clock, wait_clock):
        nc = self.nc
        carrier = nc.sync.nop(hint="drain_wait_carrier", nofuse=True)
        wait_clock.add_sem_waits(
            carrier.ins, ScopedClock({None: tick_clock.global_clock})
        )
        si = carrier.ins.sync_info
        waits = list(si.on_wait) if si is not None else []
        if len(waits) > 1:
            carrier.ins.sync_info = mybir.SyncInfo(
                on_wait=waits[:1], on_update=list(si.on_update)
            )
            for i in range(1, len(waits)):
                extra = nc.sync.nop(hint=f"drain_wait_{i}", nofuse=True)
                extra.ins.sync_info = mybir.SyncInfo(
                    on_wait=waits[i:i + 1], on_update=[]
                )
        nc.sync.drain()
        nc.all_engine_barrier()
        assert self.sems is not None
        popped = nc._tile_sem_poison_stack.pop()
        assert popped is self._sem_poison
        nc.clear_and_free_semaphores(list(self.sems.allocated().values()))
        nc.all_engine_barrier()


def _build_nc():
    f16, f32 = mybir.dt.float16, mybir.dt.float32
    i16 = mybir.dt.int16
    nc = bacc.Bacc()

    win_in = nc.declare_dram_parameter(
        "win", [SHARDS_PER_CORE, WIN_ROWS, ELEM], f16, isOutput=False)
    idx_in = nc.declare_dram_parameter(
        "idx", [SHARDS_PER_CORE, TILES_PER_SHARD, 128, IDXW], i16, isOutput=False)
    wts_in = nc.declare_dram_parameter(
        "wts", [C, K, 2, C], f16, isOutput=False)
    gb_in = nc.declare_dram_parameter("gb", [128, 3], f32, isOutput=False)
    out_ext = nc.declare_dram_parameter("out", [C, OUT_COLS], f32, isOutput=True)

    part_dram = nc.dram_tensor("stat_partial", [128, 2], f32)
    allred_dram = nc.dram_tensor("stat_total", [128, 2], f32, addr_space="Shared")

    n_tiles = SHARDS_PER_CORE * TILES_PER_SHARD  # 50

    with _SplitDrainTileContext(nc) as tc:
        with (
            tc.tile_pool(name="const", bufs=1) as cpool,
            tc.tile_pool(name="idxp", bufs=3) as idxp,
            tc.tile_pool(name="gat", bufs=3) as gatp,
            tc.tile_pool(name="stage", bufs=3) as stagep,
            tc.tile_pool(name="psum", bufs=2, space="PSUM") as psump,
        ):
            w_t = cpool.tile([C, K, 2, C], f16)
            nc.sync.dma_start(out=w_t[:], in_=wts_in[:])
            gb_t = cpool.tile([128, 3], f32)
            nc.sync.dma_start(out=gb_t[:], in_=gb_in[:])
            sums = cpool.tile([128, n_tiles], f32)
            sumsqs = cpool.tile([128, n_tiles], f32)
            conv_sb = cpool.tile([C, OUT_COLS], f32)  # SBUF-resident conv out

            # ---------------- phase A: conv, per 512-row tile ----------------
            for t in range(n_tiles):
                s, ts_ = divmod(t, TILES_PER_SHARD)
                idx_t = idxp.tile([128, IDXW], i16, tag="idx")
                nc.sync.dma_start(out=idx_t[:], in_=idx_in[s, ts_])
                ps = psump.tile([C, TILE], f32, tag="ps")
                n_mm = MM_TERMS * K
                i = 0
                KPG = K // GATHERS_PER_TILE  # offsets per gather
                GIDX = KPG * TILE            # idx per gather (<=~5400 ring cap)
                for b in range(GATHERS_PER_TILE):
                    # batched gather: KPG offsets x 512 rows in one SWDGE op
                    g_t = gatp.tile([128, 2, GIDX], f16, tag="g")
                    if not SKIP_GATHER:
                        nc.gpsimd.dma_gather(
                            out_ap=g_t[:], in_ap=win_in[s],
                            idxs_ap=idx_t[:, b * (GIDX // 16):(b + 1) * (GIDX // 16)],
                            num_idxs=GIDX, num_idxs_reg=GIDX, elem_size=ELEM,
                            transpose=True, single_packet=False,
                        )
                    elif t == 0 and b == 0:
                        nc.vector.memset(g_t[:], 0.0)
                    for q in range(KPG):
                        k = b * KPG + q
                        ghi = g_t[:, 0, q * TILE:(q + 1) * TILE]
                        glo = g_t[:, 1, q * TILE:(q + 1) * TILE]
                        whi = w_t[:, k, 0, :]
                        wlo = w_t[:, k, 1, :]
                        terms = ((whi, ghi), (whi, glo), (wlo, ghi))[:MM_TERMS]
                        if SKIP_MM:
                            terms = ()
                            if i == 0:
                                nc.tensor.matmul(out=ps[:], lhsT=whi, rhs=ghi,
                                                 start=True, stop=True)
                            i = 1
                        for lhsT, rhs in terms:
                            nc.tensor.matmul(out=ps[:], lhsT=lhsT, rhs=rhs,
                                             start=(i == 0), stop=(i == n_mm - 1))
                            i += 1
                sq_sb = stagep.tile([C, TILE], f32, tag="sq")
                nc.scalar.activation(
                    out=conv_sb[:, t * TILE:(t + 1) * TILE], in_=ps[:],
                    func=mybir.ActivationFunctionType.Copy,
                    accum_out=sums[:, t:t + 1])
                nc.scalar.activation(
                    out=sq_sb[:], in_=ps[:],
                    func=mybir.ActivationFunctionType.Square,
                    accum_out=sumsqs[:, t:t + 1])

            # ---------------- BN stats + all-reduce ----------------
            part = cpool.tile([128, 2], f32)
            nc.vector.reduce_sum(part[:, 0:1], sums[:], axis=mybir.AxisListType.X)
            nc.vector.reduce_sum(part[:, 1:2], sumsqs[:], axis=mybir.AxisListType.X)
            nc.sync.dma_start(out=part_dram[:], in_=part[:])
            nc.gpsimd.collective_compute(
                "AllReduce", mybir.AluOpType.add,
                replica_groups=[list(range(NCORES))],
                ins=[part_dram[:]], outs=[allred_dram[:]],
            )
            tot = cpool.tile([128, 2], f32)
            nc.sync.dma_start(out=tot[:], in_=allred_dram[:])

            mean = cpool.tile([128, 1], f32)
            e2 = cpool.tile([128, 1], f32)
            var = cpool.tile([128, 1], f32)
            sd = cpool.tile([128, 1], f32)
            rstd = cpool.tile([128, 1], f32)
            scale = cpool.tile([128, 1], f32)
            shift = cpool.tile([128, 1], f32)
            nc.scalar.mul(out=mean[:], in_=tot[:, 0:1], mul=1.0 / N)
            nc.scalar.mul(out=e2[:], in_=tot[:, 1:2], mul=1.0 / N)
            nc.vector.tensor_tensor(out=var[:], in0=mean[:], in1=mean[:],
                                    op=mybir.AluOpType.mult)
            nc.vector.tensor_tensor(out=var[:], in0=e2[:], in1=var[:],
                                    op=mybir.AluOpType.subtract)
            nc.scalar.activation(out=sd[:], in_=var[:],
                                 func=mybir.ActivationFunctionType.Sqrt,
                                 bias=gb_t[:, 2:3])
            nc.vector.reciprocal(out=rstd[:], in_=sd[:])
            nc.vector.tensor_tensor(out=scale[:], in0=gb_t[:, 0:1], in1=rstd[:],
                                    op=mybir.AluOpType.mult)
            nc.vector.tensor_tensor(out=shift[:], in0=mean[:], in1=scale[:],
                                    op=mybir.AluOpType.mult)
            nc.vector.tensor_tensor(out=shift[:], in0=gb_t[:, 1:2], in1=shift[:],
                                    op=mybir.AluOpType.subtract)

            # ---------------- phase B: apply relu(scale*x + shift) ----------
            for t in range(0 if not SKIP_PHASE_B else n_tiles, n_tiles):
                fbuf = stagep.tile([C, TILE], f32, tag="fbuf")
                nc.scalar.activation(
                    out=fbuf[:], in_=conv_sb[:, t * TILE:(t + 1) * TILE],
                    func=mybir.ActivationFunctionType.Relu,
                    scale=scale[:, 0:1], bias=shift[:, 0:1])
                nc.sync.dma_start(
                    out=out_ext[:, t * TILE:(t + 1) * TILE], in_=fbuf[:])

    nc.finalize()
    return nc


def _get_nc():
    if "nc" not in _COMPILED:
        _COMPILED["nc"] = _build_nc()
    return _COMPILED["nc"]


# ------------------------------------------------------------ host side
def _rcm_order(nbr_idx):
    import scipy.sparse as sp
    from scipy.sparse.csgraph import reverse_cuthill_mckee

    rows, cols = [], []
    for k in range(K):
        if k == K // 2:
            continue
        idx = nbr_idx[k]
        m = idx >= 0
        rows.append(np.nonzero(m)[0])
        cols.append(idx[m])
    r = np.concatenate(rows)
    c = np.concatenate(cols)
    A = sp.coo_matrix((np.ones(r.size, dtype=np.int8), (r, c)),
                      shape=(N, N)).tocsr()
    perm = np.asarray(reverse_cuthill_mckee(A, symmetric_mode=True),
                      dtype=np.int64)
    return perm


def _prepare(features, nbr_idx, W, gamma, beta):
    features = np.ascontiguousarray(np.asarray(features, dtype=np.float32))
    nbr_idx = np.ascontiguousarray(np.asarray(nbr_idx, dtype=np.int32))
    W = np.asarray(W, dtype=np.float32)
    gamma = np.asarray(gamma, dtype=np.float32)
    beta = np.asarray(beta, dtype=np.float32)

    perm = _rcm_order(nbr_idx)
    inv = np.empty(N, dtype=np.int64)
    inv[perm] = np.arange(N)
    # rulebook in the new ordering: row i (new) offset k -> new src index
    nbr_new = np.where(nbr_idx >= 0, inv[np.maximum(nbr_idx, 0)], -1)[:, perm]

    feats_new = features[perm]
    hi = feats_new.astype(np.float16)
    lo = (feats_new - hi.astype(np.float32)).astype(np.float16)
    tab = np.concatenate([hi, lo], axis=1)  # [N, 256] f16

    # per-shard windows + wrapped int16 indices
    wins = np.zeros((SHARDS, WIN_ROWS, ELEM), dtype=np.float16)
    idxs = np.empty((SHARDS, TILES_PER_SHARD, 128, IDXW), dtype=np.int16)
    for s in range(SHARDS):
        r0, r1 = s * PER_SHARD, (s + 1) * PER_SHARD
        sl = nbr_new[:, r0:r1]                      # [27, 12500]
        valid = sl >= 0
        lo_s = int(sl[valid].min())
        width = int(sl[valid].max()) - lo_s + 1
        assert width <= WIN_ROWS - 1, (s, width)
        wins[s, :min(width, N - lo_s)] = tab[lo_s:lo_s + width]
        loc = np.full((K, PAD_SHARD), ZERO_IDX, dtype=np.int64)
        loc[:, :PER_SHARD] = np.where(valid, sl - lo_s, ZERO_IDX)
        # tiles: [27, 25, 512] -> per tile flatten k-major -> wrap 16
        loc = loc.reshape(K, TILES_PER_SHARD, TILE).transpose(1, 0, 2)
        flat = loc.reshape(TILES_PER_SHARD, NIDX)
        wrapped = flat.reshape(TILES_PER_SHARD, IDXW, 16).transpose(0, 2, 1)
        idxs[s] = np.tile(wrapped, (1, 8, 1)).astype(np.int16)

    Whi = W.astype(np.float16)
    Wlo = (W - Whi.astype(np.float32)).astype(np.float16)
    # [K,Cin,Cout] x2 -> [Cin, K, 2, Cout]
    wts = np.stack([Whi, Wlo], axis=1).transpose(2, 0, 1, 3).copy()
    gb = np.stack([gamma, beta, np.full(C, BN_EPS, np.float32)],
                  axis=1).astype(np.float32)

    in_maps = []
    for core in range(NCORES):
        s0 = core * SHARDS_PER_CORE
        in_maps.append({
            "win": wins[s0:s0 + SHARDS_PER_CORE],
            "idx": idxs[s0:s0 + SHARDS_PER_CORE],
            "wts": wts,
            "gb": gb,
        })
    return in_maps, perm


def _assemble(results, perm):
    out_T = np.empty((C, N), dtype=np.float32)
    for s in range(SHARDS):
        core, j = divmod(s, SHARDS_PER_CORE)
        block = results[core]["out"][:, j * PAD_SHARD:
                                     j * PAD_SHARD + PER_SHARD]
        out_T[:, s * PER_SHARD:(s + 1) * PER_SHARD] = block
    out_new = out_T.T  # [N, C] in RCM order
    out = np.empty((N, C), dtype=np.float32)
    out[perm] = out_new
    return out


def _numpy_fallback(features, nbr_idx, W, gamma, beta):
    out = np.zeros((N, C), dtype=np.float64)
    for k in range(K):
        idx = nbr_idx[k]
        g = np.where((idx >= 0)[:, None], features[np.maximum(idx, 0)], 0.0)
        out += g.astype(np.float64) @ W[k].astype(np.float64)
    mean = out.mean(0)
    var = ((out - mean) ** 2).mean(0)
    out = (out - mean) * (gamma / np.sqrt(var + BN_EPS)) + beta
    return np.maximum(out, 0.0).astype(np.float32)


def kernel(features, nbr_idx, W, gamma, beta):
    try:
        in_maps, perm = _prepare(features, nbr_idx, W, gamma, beta)
    except AssertionError:
        # rulebook without enough spatial structure for int16 windows
        # (never the case for the real voxel-grid inputs)
        print("kernel: window overflow, using host fallback", file=sys.stderr)
        return _numpy_fallback(
            np.asarray(features, np.float32), np.asarray(nbr_idx),
            np.asarray(W, np.float32), np.asarray(gamma, np.float32),
            np.asarray(beta, np.float32))
    nc = _get_nc()
    res = run_bass_kernel_spmd(nc, in_maps, core_ids=list(range(NCORES)))
    return _assemble(res.results, perm)


def time_hw(inputs, reps=5):
    """Min wall-clock of one full 8-core NEFF execution with device-resident
    inputs (excludes host prep, input shipping, and compile)."""
    import time as _time

    import jax
    import jax.numpy as jnp
    from jax.sharding import Mesh, NamedSharding, PartitionSpec

    from concourse import bass2jax, mybir as _mb

    in_maps, _ = _prepare(**inputs)
    nc = _get_nc()
    bass2jax.install_neuronx_cc_hook()

    partition_name = (nc.partition_id_tensor.name
                      if nc.partition_id_tensor else None)
    in_names, out_names, out_avals = [], [], []
    for alloc in nc.m.functions[0].allocations:
        if not isinstance(alloc, _mb.MemoryLocationSet):
            continue
        name = alloc.memorylocations[0].name
        if alloc.kind == "ExternalInput":
            if name != partition_name:
                in_names.append(name)
        elif alloc.kind == "ExternalOutput":
            out_names.append(name)
            out_avals.append(jax.core.ShapedArray(
                tuple(alloc.tensor_shape), _mb.dt.np(alloc.dtype)))

    all_in_names = list(in_names) + list(out_names)
    if partition_name is not None:
        all_in_names.append(partition_name)

    def _body(*args):
        ops = list(args)
        if partition_name is not None:
            ops.append(bass2jax.partition_id_tensor())
        return tuple(bass2jax._bass_exec_p.bind(
            *ops,
            out_avals=tuple(out_avals),
            in_names=tuple(all_in_names),
            out_names=tuple(out_names),
            lowering_input_output_aliases=(),
            sim_require_finite=True,
            sim_require_nnan=True,
            nc=nc,
        ))

    devices = jax.devices()[:NCORES]
    mesh = Mesh(np.asarray(devices), ("core",))
    from jax.experimental.shard_map import shard_map
    n_args = len(in_names) + len(out_avals)
    donate = tuple(range(len(in_names), n_args))
    sharded = jax.jit(shard_map(
        _body, mesh=mesh,
        in_specs=(PartitionSpec("core"),) * n_args,
        out_specs=(PartitionSpec("core"),) * len(out_names),
        check_rep=False), donate_argnums=donate, keep_unused=True)

    sh = NamedSharding(mesh, PartitionSpec("core"))
    dev_in = [
        jax.device_put(
            np.concatenate([np.asarray(in_maps[c][n]) for c in range(NCORES)],
                           axis=0), sh)
        for n in in_names
    ]

    def _zeros():
        return [
            jax.device_put(
                np.zeros((NCORES * av.shape[0], *av.shape[1:]), av.dtype), sh)
            for av in out_avals
        ]

    r = sharded(*dev_in, *_zeros())
    jax.block_until_ready(r)
    best = float("inf")
    for _ in range(reps):
        z = _zeros()
        jax.block_until_ready(z)
        t0 = _time.perf_counter()
        r = sharded(*dev_in, *z)
        jax.block_until_ready(r)
        best = min(best, _time.perf_counter() - t0)
    return best * 1e9
